# revision 1
# baseline (speedup 1.0000x reference)
"""Trainium2 Bass kernel for nn_AtenMmQuint8: quint8 dense matmul.

    out = ((x - 65) * 0.199) @ ((y - 160) * 0.0215)
    x: [2048, 4096] int32 (quint8 values 0..255)
    y: [4096, 2048] int32 (quint8 values 0..255)
    out: [2048, 2048] fp32

Sharding: 4x2 tensor-parallel grid over the 8 NeuronCores (4 M-blocks x
2 N-blocks). This halves per-core HBM traffic vs. the 1x8 column-only
split, and keeps each core's matmul work identical (256 PE matmuls).

Host staging: the inputs are quint8 tensors boxed in int32; we stage them
to the device in their natural 1-byte storage, and stage x K-major
(transposed) so the PE's stationary operand needs no on-chip transpose
(DMA transpose only supports 2-byte dtypes).

Device kernel (identical SPMD program on all 8 cores):
  - K is interleaved across SBUF partitions (k = p*32 + j) so each
    load-chunk DMA is 128 large contiguous runs (one per partition)
    instead of 128*nk sub-1KB ones; HWDGE descriptor generation is
    ~4ns/descriptor, so the naive layout serializes the whole load
    stream. The contraction is a permutation of K applied identically
    to both operands, so the matmul result is unchanged.
  - Progressively-sized load chunks (x on the SP HWDGE ring, y on the
    ACT ring, in parallel) into persistent u8 SBUF buffers; per-ring DMA
    completions serialize at ~2.2us each, so small leading chunks start
    the pipeline and big trailing chunks amortize.
  - Dequant bias casts on VectorE: bf16 <- u8 + (-zero_point).
    (q - zp) is an integer in [-160, 190] -> exactly representable in
    bf16, so the matmul is exact to fp32 accumulation order.
  - PE prewarm: ~30 throwaway matmuls while the first loads are in
    flight release the HAM clock gate (1.2 -> 2.4 GHz) just as the real
    stream starts.
  - PE matmul bf16 x bf16 -> fp32 at the 215ns/matmul roofline,
    accumulating the whole 512x1024 block across all 8 PSUM banks
    k-outer (PE never waits on a full K pass); the last 8 k-tiles run
    m-major so banks retire early and their copy+store overlaps the
    remaining matmuls.
  - Scale+copy PSUM -> SBUF fused with the combined scale on VectorE,
    one store DMA per 128-row group (the last split in two so the
    kernel-ending chain is short).
"""

import numpy as np

import concourse.bass as bass  # noqa: F401  (kept for callers/debugging)
import concourse.mybir as mybir
import concourse.tile as tile
from concourse import bacc
from concourse.bass_utils import run_bass_kernel_spmd

X_ZP, Y_ZP = 65.0, 160.0
SCALE = 0.199 * 0.0215

M, K, N = 2048, 4096, 2048
GM, GN = 4, 2  # core grid: 4 M-blocks x 2 N-blocks
MC, NC = M // GM, N // GN  # 512 x 1024 per-core output block
P = 128  # partitions / k-tile size
NB = 512  # psum bank free size (one fp32 bank; matmul cannot cross banks)
# k-tiles per load DMA and per dequant-cast op. Per-ring DMA
# completions serialize at ~2.2us, so small leading chunks start the
# pipeline early and big trailing chunks amortize; cast ranges nest
# inside DMA ranges so a cast waits on exactly one transfer.
DMA_CHUNKS = (1, 1, 2, 4, 8, 8, 8)
SW_BULK = 0  # SWDGE bulk path measured slower; disabled
CAST_CHUNKS = (1, 1, 2, 4, 4, 4, 4, 4, 4, 4)
KT_TAIL = 8  # trailing k-tiles run m-major so PSUM banks retire early
N_WARM = 30


def _emit(tc, xT, ys, out, dma_chunks=DMA_CHUNKS, cast_chunks=CAST_CHUNKS,
          kt_tail=KT_TAIL, n_warm=N_WARM, sw_bulk=SW_BULK):
    """Emit the per-core device program.

    xT: [k, mc] u8 DRAM (x slice, K-major), ys: [k, nnc] u8 DRAM,
    out: [mc, nnc] fp32 DRAM.
    """
    nc = tc.nc
    k, mc = xT.shape
    nnc = ys.shape[1]
    kt = k // P
    mt = mc // P
    nt = nnc // NB
    assert sum(dma_chunks) + sw_bulk == kt and sum(cast_chunks) == kt

    fp32 = mybir.dt.float32
    bf16 = mybir.dt.bfloat16
    u8 = mybir.dt.uint8

    with (
        tc.tile_pool(name="sb", bufs=1) as sbp,
        tc.tile_pool(name="osb", bufs=mt, space="SBUF") as osbp,
        tc.tile_pool(name="ps", bufs=mt * nt, space="PSUM") as psp,
    ):
        # Everything is persistent (fits in SBUF at this problem size):
        # each DMA / cast writes a disjoint slice, so instructions don't
        # accrue buffer-recycling waits.
        xu = sbp.tile([P, kt, mc], u8, name="xu")
        yu = sbp.tile([P, kt, nnc], u8, name="yu")
        xba = sbp.tile([P, kt, mc], bf16, name="xba")
        yba = sbp.tile([P, kt, nnc], bf16, name="yba")
        wt = sbp.tile([P, P], bf16, name="wt")
        psum = [
            [psp.tile([P, NB], fp32, tag="ps", name=f"ps_{m}_{n}") for n in range(nt)]
            for m in range(mt)
        ]

        # HAM prewarm: the PE sits idle for ~4 us while the first chunks
        # load+cast; throwaway matmuls release the clock gate to 8/8
        # before the real stream starts.
        nc.gpsimd.memset(wt[:], 0.0)
        for _ in range(n_warm):
            nc.tensor.matmul(psum[0][0][:, :P], wt[:], wt[:], start=True, stop=True)

        # K is interleaved across partitions (k = p*kt + j): each
        # partition's j-range is then one big contiguous DRAM run, so a
        # chunk DMA is 128 descriptors (one per partition) instead of
        # 128*nk 0.5-1KB ones -- HWDGE descriptor generation (~4ns/desc)
        # otherwise serializes the whole load stream. The contraction is
        # a permutation of K, identical for x and y, so the matmul sum
        # is unchanged.
        xTr = xT.rearrange("(p j) m -> p j m", j=kt)
        ysr = ys.rearrange("(p j) n -> p j n", j=kt)
        # Bulk tail of the loads rides SWDGE (gpsimd) -- a third DMA path
        # running in parallel with both HWDGE rings. Its slower software
        # issue/completion doesn't matter for data the PE only needs
        # ~25us later, and it keeps the HWDGE rings' ~2.2us/DMA
        # completion slots for the latency-critical early chunks.
        if sw_bulk:
            sw0 = kt - sw_bulk
            nc.gpsimd.dma_start(yu[:, sw0:kt, :], ysr[:, sw0:kt, :])
            nc.gpsimd.dma_start(xu[:, sw0:kt, :], xTr[:, sw0:kt, :])
        k0 = 0
        for nk in dma_chunks:
            nc.sync.dma_start(xu[:, k0 : k0 + nk, :], xTr[:, k0 : k0 + nk, :])
            # y-loads issue from the ACT HWDGE ring, in parallel with the
            # x-load issues on the SP ring.
            nc.scalar.dma_start(yu[:, k0 : k0 + nk, :], ysr[:, k0 : k0 + nk, :])
            k0 += nk

        k0 = 0
        for nk in cast_chunks:
            sl = slice(k0, k0 + nk)
            nc.vector.tensor_scalar_add(xba[:, sl, :], xu[:, sl, :], -X_ZP)
            nc.vector.tensor_scalar_add(yba[:, sl, :], yu[:, sl, :], -Y_ZP)
            k0 += nk

        def mm(j, m, n):
            nc.tensor.matmul(
                psum[m][n][:],
                xba[:, j, m * P : (m + 1) * P],
                yba[:, j, n * NB : (n + 1) * NB],
                start=(j == 0),
                stop=(j == kt - 1),
            )

        # k-outer: touch every psum bank each k-tile so the PE stream
        # stays dense while loads/casts race ahead.
        for j in range(kt - kt_tail):
            for m in range(mt):
                for n in range(nt):
                    mm(j, m, n)
        # m-outer tail: bank group m finishes its K accumulation early so
        # its copy+store overlaps the remaining matmuls.
        for m in range(mt):
            for j in range(kt - kt_tail, kt):
                for n in range(nt):
                    mm(j, m, n)

        # Scale+copy PSUM->SBUF on VectorE (ACT stays DMA-issue only, no
        # activation-table load), one store per 128-row group (contiguous
        # rows of `out`).
        for m in range(mt):
            osb = osbp.tile([P, nnc], fp32, tag="osb", name=f"osb_{m}")
            for n in range(nt):
                nc.vector.tensor_scalar_mul(
                    osb[:, n * NB : (n + 1) * NB], psum[m][n][:], SCALE
                )
            if m < mt - 1:
                nc.sync.dma_start(out[m * P : (m + 1) * P, :], osb[:])
            else:
                # split the last row-group's store so the kernel-ending
                # chain (last matmul -> copy -> store) is half as long
                for n in range(nt):
                    nc.sync.dma_start(
                        out[m * P : (m + 1) * P, n * NB : (n + 1) * NB],
                        osb[:, n * NB : (n + 1) * NB],
                    )


def _build_nc(k=K, mc=MC, nnc=NC, **emit_kw):
    nc = bacc.Bacc("TRN2", target_bir_lowering=False, debug=False)
    xT = nc.declare_dram_parameter("xT", [k, mc], mybir.dt.uint8, isOutput=False)
    ys = nc.declare_dram_parameter("ys", [k, nnc], mybir.dt.uint8, isOutput=False)
    out = nc.declare_dram_parameter("out", [mc, nnc], mybir.dt.float32, isOutput=True)
    with tile.TileContext(nc) as tc:
        _emit(tc, xT[:], ys[:], out[:], **emit_kw)
    nc.compile()
    return nc


_CACHE = {}


def _get_nc():
    if "nc" not in _CACHE:
        _CACHE["nc"] = _build_nc()
    return _CACHE["nc"]


def kernel(x, y):
    x = np.asarray(x)
    y = np.asarray(y)
    assert x.shape == (M, K) and y.shape == (K, N)
    # quint8 payload boxed in int32 (guaranteed 0..255 by the problem spec);
    # stage in natural 1-byte storage, x in K-major layout.
    xT_u8 = x.T.astype(np.uint8)
    y_u8 = y.astype(np.uint8)

    in_maps = []
    for i in range(GM * GN):
        mi, ni = divmod(i, GN)
        in_maps.append(
            {
                "xT": np.ascontiguousarray(xT_u8[:, mi * MC : (mi + 1) * MC]),
                "ys": np.ascontiguousarray(y_u8[:, ni * NC : (ni + 1) * NC]),
            }
        )

    res = run_bass_kernel_spmd(_get_nc(), in_maps, list(range(GM * GN)))
    _CACHE["last_results"] = res

    out = np.empty((M, N), np.float32)
    for i in range(GM * GN):
        mi, ni = divmod(i, GN)
        out[mi * MC : (mi + 1) * MC, ni * NC : (ni + 1) * NC] = res.results[i]["out"]
    return out

